# Initial kernel scaffold
#
"""LIF (leaky integrate-and-fire) scan over trailing time axis, per-timestep
spike counts, on 8 Trainium2 NeuronCores.

Input:  X [64, 128, 128, 64] fp32  (last axis = time, T=64)
Output: [64] fp32 — per-timestep sum of spikes over all spatial elements.

Recurrence per spatial element (DECAY=0.5, THRESH=1.0):
    mem = mem*0.5 + x_t;  s = (mem >= 1);  mem = mem*(1-s);  out[t] += s

Strategy:
  - Data-parallel shard over the leading batch dim: 8 cores x [8,128,128,64].
  - Per core, view the shard as [128 partitions, 1024 spatial, 64 time]
    (zero-copy reshape; each partition's DRAM span is contiguous).
  - One custom DVE instruction per timestep does the WHOLE step for a
    [128, S2] slab: decode previous encoded membrane, decay+add, threshold,
    re-encode, and (via the accum path) fold the output over the free dim.
    Spikes are encoded by adding SENT=2^20 to the membrane value, so the
    per-partition fold equals SENT*spike_count + sum(mem), and the host
    recovers exact integer counts with round(fold/SENT).
  - DMA in is fully contiguous per partition; counts out are tiny.
"""

import os

import numpy as np

T = 64  # time steps (trailing axis)
S2 = 256  # spatial elements per partition per tile
NSPATIAL = 1024  # spatial elements per partition per core (8*128*128/128)
NT = NSPATIAL // S2  # tiles per core
N_CORES = 8
SENT = float(2.0**20)  # spike sentinel added to membrane
DECAY = 0.5
THRESH = 1.0

_OP_NAME = "LIF_STEP_ANT"

# populated by test.py via trace runs
last_exec_time_ns = None
last_results = None


def _register_lif_op():
    """Register the fused LIF-step custom DVE op (idempotent).

    body (per element, enc = encoded membrane stream):
        d   = enc < 1            # 0 iff previous step spiked (enc >= 1+SENT-ish)
        m   = enc * d            # decoded membrane (reset applied)
        u   = m * 0.5 + x        # decay + integrate
        s   = u >= 1             # spike
        out = u + s * SENT       # re-encode
    accum_out = sum(out) over free dim = SENT*count + sum(u)  (|sum(u)| << SENT/2)
    """
    from operator import add

    from concourse import dve_ops
    from concourse.dve_spec import C0, C1, C2, Spec, Src0, Src1, lower
    from concourse.dve_uop import DveOpSpec

    for o in dve_ops.OPS:
        if o.name == _OP_NAME:
            return o

    d = Src0 < C1
    m = Src0 * d
    u = m * C0 + Src1
    s = u >= C1
    body = u + s * C2

    def _lif_ref(in0, in1, s0, s1, imm2):
        in0 = in0.astype(np.float32)
        dd = (in0 < s1).astype(np.float32)
        uu = ((in0 * dd) * np.float32(s0) + in1).astype(np.float32)
        ss = (uu >= s1).astype(np.float32)
        b = (uu + ss * np.float32(imm2)).astype(np.float32)
        acc = b.reshape(b.shape[0], -1).sum(axis=-1, keepdims=True)
        return b, acc.astype(np.float32)

    spec = Spec(body=body, accum=add, reference=_lif_ref)
    row = dve_ops._CUSTOM_DVE_ROW_BASE + len(dve_ops.OPS)
    dve_ops._SUB_OPCODE_FOR_NAME[_OP_NAME] = row
    shas = {}
    for ver in ("v3", "v4"):
        uops = lower(spec, ver=ver)
        shas[ver] = DveOpSpec(
            name=_OP_NAME, opcode=row, uops=uops, rd1_en=True
        ).sha(ver)
    op = dve_ops.DveOp(_OP_NAME, spec, subdim=False, uops_sha=shas)
    dve_ops.OPS.append(op)
    dve_ops.CUSTOM_DVE_SPECS[_OP_NAME] = op.spec
    return op


def build_bass(nspatial=NSPATIAL, s2=S2, t=T):
    """Build the per-core Bass module (SPMD: same program on all cores)."""
    import concourse.bass as bass
    import concourse.mybir as mybir
    import concourse.tile as tile

    op = _register_lif_op()
    nt = nspatial // s2
    fp32 = mybir.dt.float32

    nc = bass.Bass(trn_type="TRN2")
    x_d = nc.dram_tensor("X", [128, nspatial, t], fp32, kind="ExternalInput")
    o_d = nc.dram_tensor("OUT", [128, nt, t], fp32, kind="ExternalOutput")

    with tile.TileContext(nc) as tc:
        with (
            tc.tile_pool(name="xp", bufs=2) as xp,
            tc.tile_pool(name="ep", bufs=2) as ep,
            tc.tile_pool(name="cp", bufs=2) as cp,
        ):
            for i in range(nt):
                xt = xp.tile([128, s2, t], fp32)
                nc.sync.dma_start(out=xt[:], in_=x_d[:, i * s2 : (i + 1) * s2, :])
                enc = ep.tile([128, 2 * s2], fp32)
                cnt = cp.tile([128, t], fp32)
                nc.gpsimd.memset(enc[:, 0:s2], 0.0)
                for k in range(t):
                    src = enc[:, (k % 2) * s2 : (k % 2 + 1) * s2]
                    dst = enc[:, ((k + 1) % 2) * s2 : ((k + 1) % 2 + 1) * s2]
                    nc.vector._custom_dve(
                        op,
                        out=dst,
                        in0=src,
                        in1=xt[:, :, k],
                        s0=DECAY,
                        s1=THRESH,
                        imm2=SENT,
                        accum_out=cnt[:, k : k + 1],
                    )
                nc.scalar.dma_start(out=o_d[:, i, :], in_=cnt[:])
    return nc


_CACHED_NC = None


def _get_nc():
    global _CACHED_NC
    if _CACHED_NC is None:
        _CACHED_NC = build_bass()
    return _CACHED_NC


def kernel(X):
    """Full-input entry point: shard over batch, run on 8 cores, unshard."""
    global last_exec_time_ns, last_results
    from concourse.bass_utils import run_bass_kernel_spmd

    X = np.asarray(X)
    assert X.shape == (64, 128, 128, 64) and X.dtype == np.float32, (
        X.shape,
        X.dtype,
    )
    nc = _get_nc()
    bs = X.shape[0] // N_CORES
    in_maps = []
    for c in range(N_CORES):
        shard = np.ascontiguousarray(X[c * bs : (c + 1) * bs]).reshape(
            128, NSPATIAL, T
        )
        in_maps.append({"X": shard})

    trace = os.environ.get("LIF_TRACE", "0") == "1"
    res = run_bass_kernel_spmd(
        nc, in_maps, core_ids=list(range(N_CORES)), trace=trace
    )
    last_exec_time_ns = res.exec_time_ns
    last_results = res
    # OUT per core: [128, NT, T] folds; recover integer counts exactly.
    total = np.zeros(T, dtype=np.float64)
    for r in res.results:
        folds = r["OUT"].astype(np.float64)
        total += np.round(folds / SENT).sum(axis=(0, 1))
    return total.astype(np.float32)


# revision 9
# speedup vs baseline: 1.0889x; 1.0889x over previous
"""LIF (leaky integrate-and-fire) scan over trailing time axis, per-timestep
spike counts, on 8 Trainium2 NeuronCores.

Input:  X [64, 128, 128, 64] fp32  (last axis = time, T=64)
Output: [64] fp32 — per-timestep sum of spikes over all spatial elements.

Recurrence per spatial element (DECAY=0.5, THRESH=1.0):
    mem = mem*0.5 + x_t;  s = (mem >= 1);  mem = mem*(1-s);  out[t] += s

Strategy:
  - Data-parallel shard over the leading batch dim: 8 cores x [8,128,128,64].
  - Per core, view the shard as [128 partitions, 1024 spatial, 64 time]
    (zero-copy reshape; each partition's DRAM span is contiguous).
  - One custom DVE instruction per timestep does the WHOLE step for a
    [128, S2] slab: decode previous encoded membrane, decay+add, threshold,
    re-encode, and (via the accum path) fold the output over the free dim.
    Spikes are encoded by adding SENT=2^20 to the membrane value, so the
    per-partition fold equals SENT*spike_count + sum(mem), and the host
    recovers exact integer counts with round(fold/SENT).
  - DMA in is fully contiguous per partition; counts out are tiny.
"""

import os

import numpy as np

T = 64  # time steps (trailing axis)
S2 = 256  # spatial elements per partition per tile
NSPATIAL = 1024  # spatial elements per partition per core (8*128*128/128)
NT = NSPATIAL // S2  # tiles per core
N_CORES = 8
SENT = float(2.0**20)  # spike sentinel added to membrane
DECAY = 0.5
THRESH = 1.0

_OP_NAME = "LIF_STEP_ANT"

# populated by test.py via trace runs
last_exec_time_ns = None
last_results = None


def _register_lif_op():
    """Register the fused LIF-step custom DVE op (idempotent).

    body (per element, enc = encoded membrane stream):
        d   = enc < 1            # 0 iff previous step spiked (enc >= 1+SENT-ish)
        m   = enc * d            # decoded membrane (reset applied)
        u   = m * 0.5 + x        # decay + integrate
        s   = u >= 1             # spike
        out = u + s * SENT       # re-encode
    accum_out = sum(out) over free dim = SENT*count + sum(u)  (|sum(u)| << SENT/2)
    """
    from operator import add

    from concourse import dve_ops
    from concourse.dve_spec import C0, C1, One, Spec, Src0, Src1, lower
    from concourse.dve_uop import DveOpSpec

    for o in dve_ops.OPS:
        if o.name == _OP_NAME:
            return o

    # threshold rides the HW constant `One` so only two scalar slots are
    # needed (s0=decay, s1=sentinel) — the TTSS encoding cannot fit
    # in0+in1+s0+s1+imm2+accum_out all at once.
    d = Src0 < One
    m = Src0 * d
    u = m * C0 + Src1
    s = u >= One
    body = u + s * C1

    def _lif_ref(in0, in1, s0, s1, imm2):
        in0 = in0.astype(np.float32)
        dd = (in0 < 1.0).astype(np.float32)
        uu = ((in0 * dd) * np.float32(s0) + in1).astype(np.float32)
        ss = (uu >= 1.0).astype(np.float32)
        b = (uu + ss * np.float32(s1)).astype(np.float32)
        acc = b.reshape(b.shape[0], -1).sum(axis=-1, keepdims=True)
        return b, acc.astype(np.float32)

    spec = Spec(body=body, accum=add, reference=_lif_ref)
    row = dve_ops._CUSTOM_DVE_ROW_BASE + len(dve_ops.OPS)
    dve_ops._SUB_OPCODE_FOR_NAME[_OP_NAME] = row
    shas = {}
    for ver in ("v3", "v4"):
        uops = lower(spec, ver=ver)
        shas[ver] = DveOpSpec(
            name=_OP_NAME, opcode=row, uops=uops, rd1_en=True
        ).sha(ver)
    op = dve_ops.DveOp(_OP_NAME, spec, subdim=False, uops_sha=shas)
    dve_ops.OPS.append(op)
    dve_ops.CUSTOM_DVE_SPECS[_OP_NAME] = op.spec
    return op


def _legalize_waits(nc, max_waits=1):
    """The walrus build in this container rejects instructions carrying more
    than one sync wait ("Too many sync wait commands" / "ISA wrong length").
    Hoist excess waits onto same-engine InstNoOps placed just before the
    offending instruction (in-order engines make this equivalent)."""
    import concourse.mybir as mybir

    n = 0
    for bb in nc.m.functions[0].blocks:
        out = []
        for ins in bb.instructions:
            si = ins.sync_info
            waits = list(si.on_wait) if si and si.on_wait else []
            if len(waits) > max_waits:
                for w in waits[max_waits:]:
                    n += 1
                    nop = mybir.InstNoOp(name=f"waitnop-{n}", engine=ins.engine)
                    nop.sync_info = mybir.SyncInfo(on_wait=[w], on_update=[])
                    out.append(nop)
                ins.sync_info = mybir.SyncInfo(
                    on_wait=waits[:max_waits], on_update=list(si.on_update or [])
                )
            out.append(ins)
        bb.instructions[:] = out
    return n


def build_bass(
    nspatial=NSPATIAL,
    s2=S2,
    t=T,
    lower=True,
    reps=1,
    tile_sizes=None,
    x_dtype="float32",
    loop_reps=0,
):
    """Build the per-core Bass module (SPMD: same program on all cores)."""
    import concourse.bass as bass
    import concourse.mybir as mybir
    import concourse.tile as tile

    op = _register_lif_op()
    if tile_sizes is None:
        tile_sizes = [s2] * (nspatial // s2)
    assert sum(tile_sizes) == nspatial, tile_sizes
    nt = len(tile_sizes)
    offs = [sum(tile_sizes[:i]) for i in range(nt)]
    fp32 = mybir.dt.float32
    xdt = getattr(mybir.dt, x_dtype)

    nc = bass.Bass(trn_type="TRN2")
    x_d = nc.dram_tensor("X", [128, nspatial, t], xdt, kind="ExternalInput")
    o_d = nc.dram_tensor("OUT", [128, nt, t], fp32, kind="ExternalOutput")

    import contextlib

    with tile.TileContext(nc) as tc:
        with (
            tc.tile_pool(name="xp", bufs=2) as xp,
            tc.tile_pool(name="ep", bufs=2) as ep,
            tc.tile_pool(name="cp", bufs=2) as cp,
            tc.For_i(0, loop_reps, 1) if loop_reps else contextlib.nullcontext(),
        ):
            for i in range(nt * reps):
                i = i % nt
                sz, off = tile_sizes[i], offs[i]
                xt = xp.tile([128, max(tile_sizes), t], xdt, tag="xt")
                nc.sync.dma_start(
                    out=xt[:, 0:sz, :], in_=x_d[:, off : off + sz, :]
                )
                enc = ep.tile([128, 2 * max(tile_sizes)], fp32, tag="enc")
                cnt = cp.tile([128, t], fp32)
                nc.gpsimd.memset(enc[:, 0:sz], 0.0)
                for k in range(t):
                    src = enc[:, (k % 2) * sz : (k % 2) * sz + sz]
                    dst = enc[:, ((k + 1) % 2) * sz : ((k + 1) % 2) * sz + sz]
                    nc.vector._custom_dve(
                        op,
                        out=dst,
                        in0=src,
                        in1=xt[:, 0:sz, k],
                        s0=DECAY,
                        s1=SENT,
                        accum_out=cnt[:, k : k + 1],
                    )
                nc.scalar.dma_start(out=o_d[:, i, :], in_=cnt[:])

    if lower:
        # plain Bass doesn't run the InstISA lowering pass (Bacc.compile
        # does); without it custom-DVE instructions serialize with zero ISA
        # bytes, and this walrus build rejects >1 sync wait per instruction.
        mybir.codegen_inst_isa_subclasses(nc)
        _legalize_waits(nc, max_waits=1)
    return nc


_CACHED_NC = None


def _get_nc():
    global _CACHED_NC
    if _CACHED_NC is None:
        _CACHED_NC = build_bass()
    return _CACHED_NC


def kernel(X):
    """Full-input entry point: shard over batch, run on 8 cores, unshard."""
    global last_exec_time_ns, last_results
    from concourse.bass_utils import run_bass_kernel_spmd

    X = np.asarray(X)
    assert X.shape == (64, 128, 128, 64) and X.dtype == np.float32, (
        X.shape,
        X.dtype,
    )
    nc = _get_nc()
    bs = X.shape[0] // N_CORES
    in_maps = []
    for c in range(N_CORES):
        shard = np.ascontiguousarray(X[c * bs : (c + 1) * bs]).reshape(
            128, NSPATIAL, T
        )
        in_maps.append({"X": shard})

    trace = os.environ.get("LIF_TRACE", "0") == "1"
    res = run_bass_kernel_spmd(
        nc, in_maps, core_ids=list(range(N_CORES)), trace=trace
    )
    last_exec_time_ns = res.exec_time_ns
    last_results = res
    # OUT per core: [128, NT, T] folds; recover integer counts exactly.
    total = np.zeros(T, dtype=np.float64)
    for r in res.results:
        folds = r["OUT"].astype(np.float64)
        total += np.round(folds / SENT).sum(axis=(0, 1))
    return total.astype(np.float32)
